# revision 1
# baseline (speedup 1.0000x reference)
"""CrossSeqAttentionLayer on 8 TRN2 NeuronCores.

Sharding: query-row split (no collectives). Core c handles batch c//2,
query rows (c%2)*1024 .. +1024, all 16 heads. Each core computes K/V for
its batch's full 2048 keys (duplicated across the 2 cores of a batch),
attention for its 1024 query rows, the output projection and layernorm
for those rows, and writes its [1024, 1024] f32 output shard.

Device-side math:
  P_unnorm^T = exp(S^T * scale) * Rfac^T      (Rfac = max(R,1e-8)**beta, host)
  attn_out^T, sumexp = (V | ones)^T @ P^T     (ones column -> denominator)
  out = LN((attn_out / sumexp) @ Wp.T + resid) * gamma + bias

Layout: S^T is computed with keys on partitions / queries on the free dim
via S^T = (K^T)^T-style matmuls; head pairs share SBUF tiles in partition
halves (rows 0-63 / 64-127) so the two K=64 matmuls run on disjoint PE
row-groups concurrently.
"""

import numpy as np
import ml_dtypes

import concourse.bass as bass
import concourse.mybir as mybir
import concourse.tile as tile
from concourse import bacc
from concourse.bass_utils import run_bass_kernel_spmd

BF16 = mybir.dt.bfloat16
F32 = mybir.dt.float32
NPBF16 = ml_dtypes.bfloat16

B, L, D = 4, 2048, 1024
H, DH = 16, 64
NCORE = 8
LQ = L // 2            # query rows per core
SCALE = DH ** -0.5
EPS_R = 1e-8
EPS_LN = 1e-5
NP = 8                 # head pairs
KB = L // 128          # key blocks of 128
DT = D // 128          # contraction tiles of 128


def _emit(tc, aps, skip_affine):
    nc = tc.nc
    (tokT, tokTq, wqp, wkp, wv, wph, rfT, resid, gamma, lnb, out) = aps

    import contextlib
    with contextlib.ExitStack() as ctx:
        t2k = ctx.enter_context(tc.tile_pool(name="t2k", bufs=58))
        wqk = ctx.enter_context(tc.tile_pool(name="wqk", bufs=32))
        vpool = ctx.enter_context(tc.tile_pool(name="vpool", bufs=16))
        expp = ctx.enter_context(tc.tile_pool(name="expp", bufs=6))
        arawp = ctx.enter_context(tc.tile_pool(name="arawp", bufs=6))
        bcp = ctx.enter_context(tc.tile_pool(name="bcp", bufs=4))
        xp = ctx.enter_context(tc.tile_pool(name="xp", bufs=2))
        smp = ctx.enter_context(tc.tile_pool(name="smp", bufs=8))
        gbp = ctx.enter_context(tc.tile_pool(name="gbp", bufs=2))
        ps_mm = ctx.enter_context(
            tc.tile_pool(name="ps_mm", bufs=2, space=bass.MemorySpace.PSUM))
        ps_pv = ctx.enter_context(
            tc.tile_pool(name="ps_pv", bufs=4, space=bass.MemorySpace.PSUM))
        drp = ctx.enter_context(
            tc.tile_pool(name="drp", bufs=4, space=bass.MemorySpace.DRAM))

        # ---- resident loads ----
        tokT_sb = []  # 16 tiles [128, 1024]: index t*2+half (t: D-tile, half: key half)
        for t in range(DT):
            for half in range(2):
                s = t2k.tile([128, 1024], BF16, tag="t2k")
                nc.sync.dma_start(out=s, in_=tokT[t * 128:(t + 1) * 128,
                                               half * 1024:(half + 1) * 1024])
                tokT_sb.append(s)
        tokTq_sb = []
        for t in range(DT):
            s = t2k.tile([128, 1024], BF16, tag="t2k")
            nc.sync.dma_start(out=s, in_=tokTq[t * 128:(t + 1) * 128, :])
            tokTq_sb.append(s)
        wv_sb = []
        for t in range(DT):
            s = t2k.tile([128, 1024], BF16, tag="t2k")
            nc.sync.dma_start(out=s, in_=wv[t * 128:(t + 1) * 128, :])
            wv_sb.append(s)

        if not skip_affine:
            gamma_b = gbp.tile([128, 1024], BF16)
            nc.gpsimd.dma_start(out=gamma_b, in_=gamma.partition_broadcast(128))
            lnb_b = gbp.tile([128, 1024], BF16)
            nc.gpsimd.dma_start(out=lnb_b, in_=lnb.partition_broadcast(128))
        eps_t = smp.tile([128, 1], F32, tag="small")
        nc.vector.memset(eps_t, EPS_LN)

        # ---- phase B: V (all heads), then per-pair K and Q ----
        vaug_sb = []  # 16 tiles [128, 16, 65]: keys-block x head x (V | ones)
        for kb in range(KB):
            va = vpool.tile([128, H, DH + 1], BF16, tag="vaug")
            for vc in range(2):
                ps = ps_mm.tile([128, 512], F32, tag="mm", name="psv", padded_shape=[128, 1024])
                for t in range(DT):
                    lhs = tokT_sb[t * 2 + kb // 8][:, (kb % 8) * 128:(kb % 8 + 1) * 128]
                    nc.tensor.matmul(ps, lhs, wv_sb[t][:, vc * 512:(vc + 1) * 512],
                                     start=(t == 0), stop=(t == DT - 1))
                psr = ps.rearrange("p (h d) -> p h d", d=DH)
                nc.vector.tensor_copy(va[:, vc * 8:(vc + 1) * 8, 0:DH], psr)
            nc.vector.memset(va[:, :, DH:DH + 1], 1.0)
            vaug_sb.append(va)

        kt_sb = [None] * (NP * 2)   # [128, 1024] per (pair, key-half)
        qt_sb = [None] * NP         # [128, 1024] per pair

        def emit_kq(p):
            wkb = [wqk.tile([128, 128], BF16, tag="wqk", name="wblk") for _ in range(DT)]
            for t in range(DT):
                nc.sync.dma_start(out=wkb[t], in_=wkp[p, t * 128:(t + 1) * 128, :])
            for half in range(2):
                for kc in range(2):
                    ps = ps_mm.tile([128, 512], F32, tag="mm", name="psk")
                    for t in range(DT):
                        rhs = tokT_sb[t * 2 + half][:, kc * 512:(kc + 1) * 512]
                        nc.tensor.matmul(ps, wkb[t], rhs,
                                         start=(t == 0), stop=(t == DT - 1))
                    if kt_sb[p * 2 + half] is None:
                        kt_sb[p * 2 + half] = t2k.tile([128, 1024], BF16,
                                                       tag="t2k", name="kt")
                    nc.vector.tensor_copy(
                        kt_sb[p * 2 + half][:, kc * 512:(kc + 1) * 512], ps)
            wqb = [wqk.tile([128, 128], BF16, tag="wqk", name="wblk") for _ in range(DT)]
            for t in range(DT):
                nc.sync.dma_start(out=wqb[t], in_=wqp[p, t * 128:(t + 1) * 128, :])
            qt_sb[p] = t2k.tile([128, 1024], BF16, tag="t2k", name="qt")
            for qc in range(2):
                ps = ps_mm.tile([128, 512], F32, tag="mm", name="psq")
                for t in range(DT):
                    nc.tensor.matmul(ps, wqb[t],
                                     tokTq_sb[t][:, qc * 512:(qc + 1) * 512],
                                     start=(t == 0), stop=(t == DT - 1))
                nc.vector.tensor_copy(qt_sb[p][:, qc * 512:(qc + 1) * 512], ps)

        rfT_sb = []
        for kb in range(KB):
            s = t2k.tile([128, 1024], BF16, tag="t2k")
            nc.sync.dma_start(out=s, in_=rfT[kb * 128:(kb + 1) * 128, :])
            rfT_sb.append(s)

        # ---- phase C: attention ----
        aoT_sb = [None] * H  # [64, 1024] bf16 per head: attn_out^T (dh x rows)
        for h in range(H):
            aoT_sb[h] = t2k.tile([64, 1024], BF16, tag="t2k", name="aoT")

        def make_norm(p, pvs):
            def go():
                srows = bcp.tile([4, 512], F32, tag="srow", name="srows")
                aors = [None] * 4
                for hi in range(2):
                    for qc in range(2):
                        i = hi * 2 + qc
                        aor = arawp.tile([DH + 1, 512], F32, tag="araw", name="aor")
                        nc.scalar.copy(aor, pvs[i])
                        nc.sync.dma_start(out=srows[i:i + 1, :],
                                          in_=aor[DH:DH + 1, :])
                        aors[i] = aor
                nc.vector.reciprocal(srows, srows)
                recd = drp.tile([4, 512], F32, tag="recd", name="recd")
                nc.sync.dma_start(out=recd, in_=srows)
                for hi in range(2):
                    h = 2 * p + hi
                    for qc in range(2):
                        i = hi * 2 + qc
                        bc = bcp.tile([DH, 512], F32, tag="bc", name="bc")
                        nc.gpsimd.dma_start(out=bc,
                                            in_=recd[i].partition_broadcast(DH))
                        nc.vector.tensor_mul(
                            aoT_sb[h][:, qc * 512:(qc + 1) * 512],
                            aors[i][0:DH, :], bc)
            return go

        pending = None
        for p in range(NP):
            emit_kq(p)
            if pending is not None:
                pending()
            pvs = [ps_pv.tile([DH + 1, 512], F32, tag="pv", name="pv") for _ in range(4)]
            for kb in range(KB):
                kt = kt_sb[p * 2 + kb // 8]
                kcol = slice((kb % 8) * 128, (kb % 8 + 1) * 128)
                va = vaug_sb[kb]
                for hi in range(2):
                    hsl = slice(hi * 64, (hi + 1) * 64)
                    sps = ps_mm.tile([128, 1024], F32, tag="mm", name="sps")
                    for qc in range(2):
                        qsl = slice(qc * 512, (qc + 1) * 512)
                        nc.tensor.matmul(sps[:, qsl], kt[hsl, kcol],
                                         qt_sb[p][hsl, qsl], start=True, stop=True)
                    e = expp.tile([128, 1024], BF16, tag="exp", name="e")
                    nc.scalar.activation(e, sps,
                                         mybir.ActivationFunctionType.Exp,
                                         scale=SCALE)
                    nc.vector.tensor_mul(e, e, rfT_sb[kb])
                    for qc in range(2):
                        qsl = slice(qc * 512, (qc + 1) * 512)
                        nc.tensor.matmul(pvs[hi * 2 + qc], va[:, 2 * p + hi, :],
                                         e[:, qsl],
                                         start=(kb == 0), stop=(kb == KB - 1))
            pending = make_norm(p, pvs)
        pending()

        # ---- phase D: projection + residual + layernorm ----
        wph_sb = []
        for h in range(H):
            s = t2k.tile([64, 1024], BF16, tag="t2k")
            nc.sync.dma_start(out=s, in_=wph[h])
            wph_sb.append(s)
        resid_sb = []
        for rb in range(8):
            s = t2k.tile([128, 1024], BF16, tag="t2k")
            nc.sync.dma_start(out=s, in_=resid[rb * 128:(rb + 1) * 128, :])
            resid_sb.append(s)

        for rb in range(8):
            rsl = slice(rb * 128, (rb + 1) * 128)
            x = xp.tile([128, 1024], F32, tag="x")
            for nch in range(2):
                nsl = slice(nch * 512, (nch + 1) * 512)
                psy = ps_mm.tile([128, 512], F32, tag="mm", name="psy")
                for h in range(H):
                    nc.tensor.matmul(psy, aoT_sb[h][:, rsl],
                                     wph_sb[h][:, nsl],
                                     start=(h == 0), stop=(h == H - 1))
                nc.vector.tensor_add(x[:, nsl], psy, resid_sb[rb][:, nsl])
            st = smp.tile([128, 2, 6], F32, tag="st")
            for s2 in range(2):
                nc.vector.bn_stats(st[:, s2, :], x[:, s2 * 512:(s2 + 1) * 512])
            mv = smp.tile([128, 2], F32, tag="mv")
            nc.vector.bn_aggr(mv, st)
            rstd = smp.tile([128, 1], F32, tag="small")
            nc.scalar.activation(rstd, mv[:, 1:2],
                                 mybir.ActivationFunctionType.Sqrt, bias=eps_t)
            nc.vector.reciprocal(rstd, rstd)
            nmr = smp.tile([128, 1], F32, tag="small")
            nc.vector.tensor_mul(nmr, mv[:, 0:1], rstd)
            nc.vector.tensor_scalar_mul(nmr, nmr, -1.0)
            nc.scalar.activation(x, x, mybir.ActivationFunctionType.Identity,
                                 bias=nmr, scale=rstd)
            if not skip_affine:
                nc.vector.tensor_mul(x, x, gamma_b)
                nc.vector.tensor_add(x, x, lnb_b)
            nc.sync.dma_start(out=out[rsl, :], in_=x)


_CACHE = {}


def _build(skip_affine):
    key = bool(skip_affine)
    if key in _CACHE:
        return _CACHE[key]
    nc = bacc.Bacc("TRN2", target_bir_lowering=False, debug=False,
                   num_devices=NCORE)
    aps = (
        nc.dram_tensor("tokT", [D, L], BF16, kind="ExternalInput").ap(),
        nc.dram_tensor("tokTq", [D, LQ], BF16, kind="ExternalInput").ap(),
        nc.dram_tensor("wqp", [NP, D, 128], BF16, kind="ExternalInput").ap(),
        nc.dram_tensor("wkp", [NP, D, 128], BF16, kind="ExternalInput").ap(),
        nc.dram_tensor("wv", [D, D], BF16, kind="ExternalInput").ap(),
        nc.dram_tensor("wph", [H, DH, D], BF16, kind="ExternalInput").ap(),
        nc.dram_tensor("rfT", [L, LQ], BF16, kind="ExternalInput").ap(),
        nc.dram_tensor("resid", [LQ, D], BF16, kind="ExternalInput").ap(),
        nc.dram_tensor("gamma", [D], BF16, kind="ExternalInput").ap(),
        nc.dram_tensor("lnb", [D], BF16, kind="ExternalInput").ap(),
        nc.dram_tensor("out", [LQ, D], F32, kind="ExternalOutput").ap(),
    )
    with tile.TileContext(nc) as tc:
        _emit(tc, aps, skip_affine)
    nc.compile()
    _CACHE[key] = nc
    return nc


def kernel(tokens, R, Wq, Wk, Wv, Wp, beta, gamma, bias, _spmd_kwargs=None):
    tokens = np.asarray(tokens, dtype=np.float32)
    R = np.asarray(R, dtype=np.float32)
    skip_affine = bool(np.all(gamma == 1.0) and np.all(bias == 0.0))
    nc = _build(skip_affine)

    rfac = np.maximum(R, EPS_R).astype(np.float64) ** float(beta[0])
    rfacT = np.ascontiguousarray(rfac.T.astype(NPBF16))
    wqT = np.ascontiguousarray(Wq.T.astype(NPBF16))
    wkT = np.ascontiguousarray(Wk.T.astype(NPBF16))
    wvT = np.ascontiguousarray(Wv.T.astype(NPBF16))
    wpT = np.ascontiguousarray(Wp.T.astype(NPBF16))
    wqp = np.ascontiguousarray(wqT.reshape(D, NP, 128).transpose(1, 0, 2))
    wkp = np.ascontiguousarray(wkT.reshape(D, NP, 128).transpose(1, 0, 2))
    wph = np.ascontiguousarray(wpT.reshape(H, DH, D))
    gamma_b = gamma.astype(NPBF16)
    lnb_b = bias.astype(NPBF16)

    in_maps = []
    for c in range(NCORE):
        b, q0 = c // 2, (c % 2) * LQ
        tokT = np.ascontiguousarray(tokens[b].T.astype(NPBF16))
        in_maps.append({
            "tokT": tokT,
            "tokTq": np.ascontiguousarray(tokT[:, q0:q0 + LQ]),
            "wqp": wqp, "wkp": wkp, "wv": wvT, "wph": wph,
            "rfT": np.ascontiguousarray(rfacT[:, q0:q0 + LQ]),
            "resid": np.ascontiguousarray(tokens[b, q0:q0 + LQ, :].astype(NPBF16)),
            "gamma": gamma_b, "lnb": lnb_b,
        })

    res = run_bass_kernel_spmd(nc, in_maps, core_ids=list(range(NCORE)),
                               **(_spmd_kwargs or {}))
    out = np.empty((B, L, D), dtype=np.float32)
    for c in range(NCORE):
        b, q0 = c // 2, (c % 2) * LQ
        out[b, q0:q0 + LQ, :] = res.results[c]["out"]
    if _spmd_kwargs:
        kernel._last_result = res
    return out



# revision 8
# speedup vs baseline: 1.4689x; 1.4689x over previous
"""CrossSeqAttentionLayer on 8 TRN2 NeuronCores — fp8 DoubleRow edition.

Sharding: query-row split (no collectives). Core c handles batch c//2,
query rows (c%2)*1024 .. +1024, all 16 heads. Keys are stored in ROTATED
order per core (query half first), so the Q projection always reads keys
0..1023; attention sums over keys, so any consistent key order works.

Q/K/V projections and the out-projection run in fp8e4m3 with the
DoubleRow perf mode (two 128-row contraction tiles per instruction, 2x PE
throughput). S = K^T Q stays bf16. P = exp(S*scale - ln64) * rfac is kept
bf16 (fast 16-bit DVE multiply); PV uses fp8 V-weights against the bf16 P.
The 1/64 exp bias keeps values small and cancels in the softmax ratio.
The V-augmented ones column is 1/16 so attn_out is stored x16 (good fp8
range); Wp is stored x16; the 1/256 is folded into the residual add.
"""

import numpy as np
import ml_dtypes
import contextlib

import concourse.bass as bass
import concourse.mybir as mybir
import concourse.tile as tile
from concourse import bacc
from concourse.bass_utils import run_bass_kernel_spmd

BF16 = mybir.dt.bfloat16
F32 = mybir.dt.float32
F8 = mybir.dt.float8e4
NPBF16 = ml_dtypes.bfloat16
NPF8 = ml_dtypes.float8_e4m3
DR = mybir.MatmulPerfMode.DoubleRow

B, L, D = 4, 2048, 1024
H, DH = 16, 64
NCORE = 8
LQ = L // 2
SCALE = DH ** -0.5
EPS_R = 1e-8
EPS_LN = 1e-5
NPAIR = 8              # head pairs
KB = 16                # key blocks of 128
TJ = 4                 # D-contraction 128x2 pairs
PBIAS = float(-np.log(64.0))
AOSCL = 16.0           # attn_out stored x16 (ones column = 1/16)
WPSCL = 16.0           # Wp stored x16
XSCL = 1.0 / (AOSCL * WPSCL)


def _emit(tc, aps, skip_affine):
    nc = tc.nc
    (tok8, wq8, wk8, wv8, wp8, rfT, resid, gamma, lnb, out) = aps

    with contextlib.ExitStack() as ctx:
        p_tok = ctx.enter_context(tc.tile_pool(name="p_tok", bufs=8))
        p_w1 = ctx.enter_context(tc.tile_pool(name="p_w1", bufs=2))
        p_wv = ctx.enter_context(tc.tile_pool(name="p_wv", bufs=4))
        p_va = ctx.enter_context(tc.tile_pool(name="p_va", bufs=16))
        p_e8 = ctx.enter_context(tc.tile_pool(name="p_e8", bufs=32))
        p_ao = ctx.enter_context(tc.tile_pool(name="p_ao", bufs=1))
        rfp = ctx.enter_context(tc.tile_pool(name="rfp", bufs=16))
        ktp = ctx.enter_context(tc.tile_pool(name="ktp", bufs=2))
        qtp = ctx.enter_context(tc.tile_pool(name="qtp", bufs=2))
        aorp = ctx.enter_context(tc.tile_pool(name="aorp", bufs=4))
        srp = ctx.enter_context(tc.tile_pool(name="srp", bufs=2))
        bcp = ctx.enter_context(tc.tile_pool(name="bcp", bufs=2))
        xp = ctx.enter_context(tc.tile_pool(name="xp", bufs=2))
        rsp = ctx.enter_context(tc.tile_pool(name="rsp", bufs=2))
        smp = ctx.enter_context(tc.tile_pool(name="smp", bufs=10))
        gbp = ctx.enter_context(tc.tile_pool(name="gbp", bufs=2))
        ps_s = ctx.enter_context(
            tc.tile_pool(name="ps_s", bufs=2, space=bass.MemorySpace.PSUM))
        ps_pv = ctx.enter_context(
            tc.tile_pool(name="ps_pv", bufs=4, space=bass.MemorySpace.PSUM))
        drp = ctx.enter_context(
            tc.tile_pool(name="drp", bufs=4, space=bass.MemorySpace.DRAM))

        # ---- resident loads, priority order ----
        tok8_sb = [[None] * TJ for _ in range(2)]  # [half][j] -> [128, 2, 1024]
        for j in range(TJ):
            src = tok8[j].rearrange("p (i k) -> p i k", i=2)
            for half in range(2):
                t = p_tok.tile([128, 2, 1024], F8, tag="tok8")
                nc.sync.dma_start(out=t,
                                  in_=src[:, :, half * 1024:(half + 1) * 1024])
                tok8_sb[half][j] = t
        wk8_sb = p_w1.tile([128, 8192], F8, tag="w1")
        wq8_sb = p_w1.tile([128, 8192], F8, tag="w1")
        for pr in range(NPAIR):
            psl = slice(pr * 1024, (pr + 1) * 1024)
            nc.sync.dma_start(out=wk8_sb[:, psl], in_=wk8[:, psl])
            nc.sync.dma_start(out=wq8_sb[:, psl], in_=wq8[:, psl])
        wk8r = wk8_sb.rearrange("p (pr j i c) -> p pr j i c", pr=NPAIR, j=TJ, i=2)
        wq8r = wq8_sb.rearrange("p (pr j i c) -> p pr j i c", pr=NPAIR, j=TJ, i=2)

        rfT_sb = []
        for kb in range(4):
            t = rfp.tile([128, 1024], BF16, tag="rf")
            nc.sync.dma_start(out=t, in_=rfT[kb * 128:(kb + 1) * 128, :])
            rfT_sb.append(t)
        wv8_sb = []
        for j in range(TJ):
            t = p_wv.tile([128, 2, 1024], F8, tag="wv8", name="wvt")
            nc.sync.dma_start(out=t, in_=wv8[j].rearrange("p (i n) -> p i n", i=2))
            wv8_sb.append(t)
        for kb in range(4, KB):
            t = rfp.tile([128, 1024], BF16, tag="rf")
            nc.sync.dma_start(out=t, in_=rfT[kb * 128:(kb + 1) * 128, :])
            rfT_sb.append(t)

        va8_sb = []  # [kb] -> [128, H, DH+1] fp8 (keys x head x V|1/16)
        for kb in range(KB):
            t = p_va.tile([128, H, DH + 1], F8, tag="va8", name="vat")
            nc.vector.memset(t[:, :, DH:DH + 1], 1.0 / AOSCL)
            va8_sb.append(t)
        e8_sb = [[None, None] for _ in range(KB)]  # [kb][hi] -> [128, 1024] bf16
        for kb in range(KB):
            for hi in range(2):
                e8_sb[kb][hi] = p_e8.tile([128, 1024], BF16, tag="e8",
                                          name="e8t")
        aoT8 = p_ao.tile([128, NPAIR, 1024], F8, tag="aoT8")

        if not skip_affine:
            gamma_b = gbp.tile([128, 1024], BF16)
            nc.gpsimd.dma_start(out=gamma_b, in_=gamma.partition_broadcast(128))
            lnb_b = gbp.tile([128, 1024], BF16)
            nc.gpsimd.dma_start(out=lnb_b, in_=lnb.partition_broadcast(128))
        eps_t = smp.tile([128, 1], F32, tag="small")
        nc.vector.memset(eps_t, EPS_LN)
        pbias_t = smp.tile([128, 1], F32, tag="small")
        nc.vector.memset(pbias_t, PBIAS)

        # ---- emit helpers ----
        def emit_kq(pr):
            kt = ktp.tile([128, L], BF16, name="kt")
            qt = qtp.tile([128, LQ], BF16, name="qt")

            def group(c):
                if c < 4:  # K chunk c (keys c*512..)
                    half, kc = c // 2, c % 2
                    ps = ps_s.tile([128, 512], F32, tag="s", name="psk", padded_shape=[128, 1024])
                    for j in range(TJ):
                        nc.tensor.matmul(
                            ps, wk8r[:, pr, j, :, :],
                            tok8_sb[half][j][:, :, kc * 512:(kc + 1) * 512],
                            perf_mode=DR, start=(j == 0), stop=(j == TJ - 1))
                    nc.vector.tensor_copy(kt[:, c * 512:(c + 1) * 512], ps)
                else:      # Q chunk (queries = keys 0..1023 by construction)
                    qc = c - 4
                    ps = ps_s.tile([128, 512], F32, tag="s", name="psq", padded_shape=[128, 1024])
                    for j in range(TJ):
                        nc.tensor.matmul(
                            ps, wq8r[:, pr, j, :, :],
                            tok8_sb[0][j][:, :, qc * 512:(qc + 1) * 512],
                            perf_mode=DR, start=(j == 0), stop=(j == TJ - 1))
                    nc.vector.tensor_copy(qt[:, qc * 512:(qc + 1) * 512], ps)
            return kt, qt, [lambda c=c: group(c) for c in range(6)]

        def emit_v(kb, vc):
            half, k8 = kb // 8, kb % 8
            ps = ps_s.tile([128, 512], F32, tag="s", name="psv", padded_shape=[128, 1024])
            for j in range(TJ):
                nc.tensor.matmul(
                    ps, tok8_sb[half][j][:, :, k8 * 128:(k8 + 1) * 128],
                    wv8_sb[j][:, :, vc * 512:(vc + 1) * 512],
                    perf_mode=DR, start=(j == 0), stop=(j == TJ - 1))
            psr = ps.rearrange("p (h d) -> p h d", d=DH)
            nc.vector.tensor_copy(va8_sb[kb][:, vc * 8:(vc + 1) * 8, 0:DH], psr)

        def emit_s(p, kb, hi, kt, qt):
            sps = ps_s.tile([128, 1024], F32, tag="s", name="sps")
            hsl = slice(hi * 64, hi * 64 + 64)
            for qc in range(2):
                qsl = slice(qc * 512, (qc + 1) * 512)
                nc.tensor.matmul(sps[:, qsl], kt[hsl, kb * 128:(kb + 1) * 128],
                                 qt[hsl, qsl], start=True, stop=True)
            e = e8_sb[kb][hi]
            nc.scalar.activation(e, sps, mybir.ActivationFunctionType.Exp,
                                 bias=pbias_t, scale=SCALE)
            nc.vector.tensor_mul(e, e, rfT_sb[kb])

        pvst = {}

        def pv_chunk(p, ck):
            # accumulate e8 kb-tiles 2ck, 2ck+1 of pair p into pvs psum
            if ck == 0:
                pvst["pvs"] = [ps_pv.tile([DH + 1, 512], F32, tag="pv",
                                          name="pv") for _ in range(4)]
            pvs = pvst["pvs"]
            for kb in (2 * ck, 2 * ck + 1):
                for hi in range(2):
                    h = 2 * p + hi
                    for qc in range(2):
                        nc.tensor.matmul(
                            pvs[hi * 2 + qc], va8_sb[kb][:, h, :],
                            e8_sb[kb][hi][:, qc * 512:(qc + 1) * 512],
                            start=(kb == 0), stop=(kb == KB - 1))

        def pv_copies():
            aors = []
            for i in range(4):
                a = aorp.tile([DH + 1, 512], F32, tag="aor", name="aor")
                nc.vector.tensor_copy(a, pvst["pvs"][i])
                aors.append(a)
            pvst["aors"] = aors

        def emit_norm(p, aors4):
            srows = srp.tile([4, 512], F32, name="srows")
            for i, a in enumerate(aors4):
                nc.sync.dma_start(out=srows[i:i + 1, :], in_=a[DH:DH + 1, :])
            nc.vector.reciprocal(srows, srows)
            recd = drp.tile([4, 512], F32, tag="recd", name="recd")
            nc.sync.dma_start(out=recd, in_=srows)
            for hi in range(2):
                for qc in range(2):
                    i = hi * 2 + qc
                    bc = bcp.tile([DH, 512], F32, tag="bc", name="bc")
                    nc.gpsimd.dma_start(out=bc,
                                        in_=recd[i].partition_broadcast(DH))
                    nc.vector.tensor_mul(
                        aoT8[hi * 64:(hi + 1) * 64, p, qc * 512:(qc + 1) * 512],
                        aors4[i][0:DH, :], bc)

        # ---- schedule ----
        vq = [(kb, vc) for kb in range(KB) for vc in range(2)]
        kt, qt, kq_groups = emit_kq(0)
        for g in kq_groups:
            g()
        wp8_sb = []
        for p in range(NPAIR):
            nxt = None
            for kb in range(KB):
                if p >= 1 and kb < 8:
                    pv_chunk(p - 1, kb)
                emit_s(p, kb, 0, kt, qt)
                emit_s(p, kb, 1, kt, qt)
                if p == 0:
                    for _ in range(2):  # V filler
                        if vq:
                            emit_v(*vq.pop(0))
                if p >= 1 and kb == 8:
                    pv_copies()
                elif p >= 1 and kb == 9:
                    emit_norm(p - 1, pvst.pop("aors"))
                if kb >= 10 and p < NPAIR - 1:
                    if nxt is None:
                        nxt = emit_kq(p + 1)
                    nxt[2][kb - 10]()
                if p == 2 and kb < 4:  # wp8 into freed wv slots after V phase
                    t = p_wv.tile([128, 2, 1024], F8, tag="wv8", name="wpt")
                    nc.sync.dma_start(
                        out=t, in_=wp8[kb].rearrange("p (i n) -> p i n", i=2))
                    wp8_sb.append(t)
            if nxt is not None:
                kt, qt = nxt[0], nxt[1]
        for ck in range(8):
            pv_chunk(NPAIR - 1, ck)
        pv_copies()
        emit_norm(NPAIR - 1, pvst.pop("aors"))

        # ---- projection + residual + layernorm ----
        resid_sb = {}

        def load_resid(rb):
            t = rsp.tile([128, 1024], BF16, tag="resid", name="rt")
            nc.sync.dma_start(out=t, in_=resid[rb * 128:(rb + 1) * 128, :])
            resid_sb[rb] = t

        load_resid(0)
        load_resid(1)
        for rb in range(8):
            rsl = slice(rb * 128, (rb + 1) * 128)
            x = xp.tile([128, 1024], F32, tag="x")
            for nch in range(2):
                nsl = slice(nch * 512, (nch + 1) * 512)
                psy = ps_s.tile([128, 512], F32, tag="s", name="psy", padded_shape=[128, 1024])
                for jq in range(4):
                    nc.tensor.matmul(psy, aoT8[:, 2 * jq:2 * jq + 2, rsl],
                                     wp8_sb[jq][:, :, nsl],
                                     perf_mode=DR, start=(jq == 0), stop=(jq == 3))
                nc.vector.scalar_tensor_tensor(
                    x[:, nsl], psy, XSCL, resid_sb[rb][:, nsl],
                    op0=mybir.AluOpType.mult, op1=mybir.AluOpType.add)
            st = smp.tile([128, 2, 6], F32, tag="st")
            for s2 in range(2):
                nc.vector.bn_stats(st[:, s2, :], x[:, s2 * 512:(s2 + 1) * 512])
            mv = smp.tile([128, 2], F32, tag="mv")
            nc.vector.bn_aggr(mv, st)
            rstd = smp.tile([128, 1], F32, tag="small")
            nc.scalar.activation(rstd, mv[:, 1:2],
                                 mybir.ActivationFunctionType.Sqrt, bias=eps_t)
            nc.vector.reciprocal(rstd, rstd)
            nmr = smp.tile([128, 1], F32, tag="small")
            nc.vector.tensor_mul(nmr, mv[:, 0:1], rstd)
            nc.vector.tensor_scalar_mul(nmr, nmr, -1.0)
            nc.scalar.activation(x, x, mybir.ActivationFunctionType.Identity,
                                 bias=nmr, scale=rstd)
            if not skip_affine:
                nc.vector.tensor_mul(x, x, gamma_b)
                nc.vector.tensor_add(x, x, lnb_b)
            nc.sync.dma_start(out=out[rsl, :], in_=x)
            if rb + 2 < 8:
                load_resid(rb + 2)


_CACHE = {}


def _build(skip_affine):
    key = bool(skip_affine)
    if key in _CACHE:
        return _CACHE[key]
    nc = bacc.Bacc("TRN2", target_bir_lowering=False, debug=False,
                   num_devices=NCORE)
    aps = (
        nc.dram_tensor("tok8", [TJ, 128, 2 * L], F8, kind="ExternalInput").ap(),
        nc.dram_tensor("wq8", [128, 8192], F8, kind="ExternalInput").ap(),
        nc.dram_tensor("wk8", [128, 8192], F8, kind="ExternalInput").ap(),
        nc.dram_tensor("wv8", [TJ, 128, 2 * D], F8, kind="ExternalInput").ap(),
        nc.dram_tensor("wp8", [TJ, 128, 2 * D], F8, kind="ExternalInput").ap(),
        nc.dram_tensor("rfT", [L, LQ], BF16, kind="ExternalInput").ap(),
        nc.dram_tensor("resid", [LQ, D], BF16, kind="ExternalInput").ap(),
        nc.dram_tensor("gamma", [D], BF16, kind="ExternalInput").ap(),
        nc.dram_tensor("lnb", [D], BF16, kind="ExternalInput").ap(),
        nc.dram_tensor("out", [LQ, D], F32, kind="ExternalOutput").ap(),
    )
    with tile.TileContext(nc) as tc:
        _emit(tc, aps, skip_affine)
    nc.compile()
    _CACHE[key] = nc
    return nc


def _dlayout(M):
    """[D, N] -> [TJ, 128, 2, N] fp8: D-dim index = j*256 + i*128 + p."""
    N = M.shape[1]
    return np.ascontiguousarray(
        M.reshape(TJ, 2, 128, N).transpose(0, 2, 1, 3).astype(NPF8))


def kernel(tokens, R, Wq, Wk, Wv, Wp, beta, gamma, bias, _spmd_kwargs=None):
    tokens = np.asarray(tokens, dtype=np.float32)
    R = np.asarray(R, dtype=np.float32)
    skip_affine = bool(np.all(gamma == 1.0) and np.all(bias == 0.0))
    nc = _build(skip_affine)

    rfac = np.maximum(R, EPS_R).astype(np.float64) ** float(beta[0])
    rfacT = rfac.T.astype(NPBF16)  # [keys, queries]
    wq8 = np.ascontiguousarray(
        Wq.T.reshape(TJ, 2, 128, NPAIR, 128).transpose(2, 3, 0, 1, 4)
        .astype(NPF8)).reshape(128, 8192)
    wk8 = np.ascontiguousarray(
        Wk.T.reshape(TJ, 2, 128, NPAIR, 128).transpose(2, 3, 0, 1, 4)
        .astype(NPF8)).reshape(128, 8192)
    wv8 = _dlayout(np.ascontiguousarray(Wv.T)).reshape(TJ, 128, 2 * D)
    # wp8 contraction index jq*256 + i*128 + p -> head 2*(2jq+i) + p//64,
    # dh = p%64, i.e. row (head*64 + dh) of Wp.T; stored x16.
    wp8 = np.ascontiguousarray(
        (Wp.T * WPSCL).reshape(TJ, 2, 2, DH, D)   # [jq, i, hpar, dh, n]
        .transpose(0, 2, 3, 1, 4)                 # [jq, hpar, dh, i, n]
        .astype(NPF8)).reshape(TJ, 128, 2 * D)
    gamma_b = gamma.astype(NPBF16)
    lnb_b = bias.astype(NPBF16)

    in_maps = []
    for c in range(NCORE):
        b, q0 = c // 2, (c % 2) * LQ
        tokT = tokens[b].T  # [D, L]
        rot = np.concatenate(
            [tokT[:, q0:q0 + LQ], tokT[:, LQ - q0:2 * LQ - q0]], axis=1)
        in_maps.append({
            "tok8": _dlayout(np.ascontiguousarray(rot)).reshape(TJ, 128, 2 * L),
            "wq8": wq8, "wk8": wk8, "wv8": wv8, "wp8": wp8,
            "rfT": np.ascontiguousarray(np.concatenate(
                [rfacT[q0:q0 + LQ, q0:q0 + LQ],
                 rfacT[LQ - q0:2 * LQ - q0, q0:q0 + LQ]], axis=0)),
            "resid": np.ascontiguousarray(
                tokens[b, q0:q0 + LQ, :].astype(NPBF16)),
            "gamma": gamma_b, "lnb": lnb_b,
        })

    res = run_bass_kernel_spmd(nc, in_maps, core_ids=list(range(NCORE)),
                               **(_spmd_kwargs or {}))
    out = np.empty((B, L, D), dtype=np.float32)
    for c in range(NCORE):
        b, q0 = c // 2, (c % 2) * LQ
        out[b, q0:q0 + LQ, :] = res.results[c]["out"]
    if _spmd_kwargs:
        kernel._last_result = res
    return out


# revision 9
# speedup vs baseline: 1.4720x; 1.0021x over previous
"""CrossSeqAttentionLayer on 8 TRN2 NeuronCores — fp8 DoubleRow edition.

Sharding: query-row split (no collectives). Core c handles batch c//2,
query rows (c%2)*1024 .. +1024, all 16 heads. Keys are stored in ROTATED
order per core (query half first), so the Q projection always reads keys
0..1023; attention sums over keys, so any consistent key order works.

Q/K/V projections and the out-projection run in fp8e4m3 with the
DoubleRow perf mode (two 128-row contraction tiles per instruction, 2x PE
throughput). S = K^T Q stays bf16. P = exp(S*scale - ln64) * rfac is kept
bf16 (fast 16-bit DVE multiply); PV uses fp8 V-weights against the bf16 P.
The 1/64 exp bias keeps values small and cancels in the softmax ratio.
The V-augmented ones column is 1/16 so attn_out is stored x16 (good fp8
range); Wp is stored x16; the 1/256 is folded into the residual add.
"""

import numpy as np
import ml_dtypes
import contextlib

import concourse.bass as bass
import concourse.mybir as mybir
import concourse.tile as tile
from concourse import bacc
from concourse.bass_utils import run_bass_kernel_spmd

BF16 = mybir.dt.bfloat16
F32 = mybir.dt.float32
F8 = mybir.dt.float8e4
NPBF16 = ml_dtypes.bfloat16
NPF8 = ml_dtypes.float8_e4m3
DR = mybir.MatmulPerfMode.DoubleRow

B, L, D = 4, 2048, 1024
H, DH = 16, 64
NCORE = 8
LQ = L // 2
SCALE = DH ** -0.5
EPS_R = 1e-8
EPS_LN = 1e-5
NPAIR = 8              # head pairs
KB = 16                # key blocks of 128
TJ = 4                 # D-contraction 128x2 pairs
PBIAS = float(-np.log(64.0))
AOSCL = 16.0           # attn_out stored x16 (ones column = 1/16)
WPSCL = 16.0           # Wp stored x16
XSCL = 1.0 / (AOSCL * WPSCL)


def _emit(tc, aps, skip_affine):
    nc = tc.nc
    (tok8, wq8, wk8, wv8, wp8, rfT, resid, gamma, lnb, out) = aps

    with contextlib.ExitStack() as ctx:
        p_tok = ctx.enter_context(tc.tile_pool(name="p_tok", bufs=8))
        p_w1 = ctx.enter_context(tc.tile_pool(name="p_w1", bufs=2))
        p_wv = ctx.enter_context(tc.tile_pool(name="p_wv", bufs=4))
        p_va = ctx.enter_context(tc.tile_pool(name="p_va", bufs=16))
        p_e8 = ctx.enter_context(tc.tile_pool(name="p_e8", bufs=32))
        p_ao = ctx.enter_context(tc.tile_pool(name="p_ao", bufs=1))
        rfp = ctx.enter_context(tc.tile_pool(name="rfp", bufs=16))
        ktp = ctx.enter_context(tc.tile_pool(name="ktp", bufs=2))
        qtp = ctx.enter_context(tc.tile_pool(name="qtp", bufs=2))
        aorp = ctx.enter_context(tc.tile_pool(name="aorp", bufs=4))
        srp = ctx.enter_context(tc.tile_pool(name="srp", bufs=2))
        bcp = ctx.enter_context(tc.tile_pool(name="bcp", bufs=2))
        xp = ctx.enter_context(tc.tile_pool(name="xp", bufs=3))
        rsp = ctx.enter_context(tc.tile_pool(name="rsp", bufs=3))
        smp = ctx.enter_context(tc.tile_pool(name="smp", bufs=10))
        gbp = ctx.enter_context(tc.tile_pool(name="gbp", bufs=2))
        ps_s = ctx.enter_context(
            tc.tile_pool(name="ps_s", bufs=2, space=bass.MemorySpace.PSUM))
        ps_pv = ctx.enter_context(
            tc.tile_pool(name="ps_pv", bufs=4, space=bass.MemorySpace.PSUM))
        drp = ctx.enter_context(
            tc.tile_pool(name="drp", bufs=4, space=bass.MemorySpace.DRAM))

        # ---- resident loads, priority order ----
        tok8_sb = [[None] * TJ for _ in range(2)]  # [half][j] -> [128, 2, 1024]
        for j in range(TJ):
            src = tok8[j].rearrange("p (i k) -> p i k", i=2)
            for half in range(2):
                t = p_tok.tile([128, 2, 1024], F8, tag="tok8")
                nc.sync.dma_start(out=t,
                                  in_=src[:, :, half * 1024:(half + 1) * 1024])
                tok8_sb[half][j] = t
        wk8_sb = p_w1.tile([128, 8192], F8, tag="w1")
        wq8_sb = p_w1.tile([128, 8192], F8, tag="w1")
        for pr in range(NPAIR):
            psl = slice(pr * 1024, (pr + 1) * 1024)
            nc.sync.dma_start(out=wk8_sb[:, psl], in_=wk8[:, psl])
            nc.sync.dma_start(out=wq8_sb[:, psl], in_=wq8[:, psl])
        wk8r = wk8_sb.rearrange("p (pr j i c) -> p pr j i c", pr=NPAIR, j=TJ, i=2)
        wq8r = wq8_sb.rearrange("p (pr j i c) -> p pr j i c", pr=NPAIR, j=TJ, i=2)

        rfT_sb = []
        for kb in range(4):
            t = rfp.tile([128, 1024], BF16, tag="rf")
            nc.sync.dma_start(out=t, in_=rfT[kb * 128:(kb + 1) * 128, :])
            rfT_sb.append(t)
        wv8_sb = []
        for j in range(TJ):
            t = p_wv.tile([128, 2, 1024], F8, tag="wv8", name="wvt")
            nc.sync.dma_start(out=t, in_=wv8[j].rearrange("p (i n) -> p i n", i=2))
            wv8_sb.append(t)
        for kb in range(4, KB):
            t = rfp.tile([128, 1024], BF16, tag="rf")
            nc.sync.dma_start(out=t, in_=rfT[kb * 128:(kb + 1) * 128, :])
            rfT_sb.append(t)

        va8_sb = []  # [kb] -> [128, H, DH+1] fp8 (keys x head x V|1/16)
        for kb in range(KB):
            t = p_va.tile([128, H, DH + 1], F8, tag="va8", name="vat")
            nc.vector.memset(t[:, :, DH:DH + 1], 1.0 / AOSCL)
            va8_sb.append(t)
        e8_sb = [[None, None] for _ in range(KB)]  # [kb][hi] -> [128, 1024] bf16
        for kb in range(KB):
            for hi in range(2):
                e8_sb[kb][hi] = p_e8.tile([128, 1024], BF16, tag="e8",
                                          name="e8t")
        aoT8 = p_ao.tile([128, NPAIR, 1024], F8, tag="aoT8")

        if not skip_affine:
            gamma_b = gbp.tile([128, 1024], BF16)
            nc.gpsimd.dma_start(out=gamma_b, in_=gamma.partition_broadcast(128))
            lnb_b = gbp.tile([128, 1024], BF16)
            nc.gpsimd.dma_start(out=lnb_b, in_=lnb.partition_broadcast(128))
        eps_t = smp.tile([128, 1], F32, tag="small")
        nc.vector.memset(eps_t, EPS_LN)
        pbias_t = smp.tile([128, 1], F32, tag="small")
        nc.vector.memset(pbias_t, PBIAS)

        # ---- emit helpers ----
        def emit_kq(pr):
            kt = ktp.tile([128, L], BF16, name="kt")
            qt = qtp.tile([128, LQ], BF16, name="qt")

            def group(c):
                if c < 4:  # K chunk c (keys c*512..)
                    half, kc = c // 2, c % 2
                    ps = ps_s.tile([128, 512], F32, tag="s", name="psk", padded_shape=[128, 1024])
                    for j in range(TJ):
                        nc.tensor.matmul(
                            ps, wk8r[:, pr, j, :, :],
                            tok8_sb[half][j][:, :, kc * 512:(kc + 1) * 512],
                            perf_mode=DR, start=(j == 0), stop=(j == TJ - 1))
                    nc.vector.tensor_copy(kt[:, c * 512:(c + 1) * 512], ps)
                else:      # Q chunk (queries = keys 0..1023 by construction)
                    qc = c - 4
                    ps = ps_s.tile([128, 512], F32, tag="s", name="psq", padded_shape=[128, 1024])
                    for j in range(TJ):
                        nc.tensor.matmul(
                            ps, wq8r[:, pr, j, :, :],
                            tok8_sb[0][j][:, :, qc * 512:(qc + 1) * 512],
                            perf_mode=DR, start=(j == 0), stop=(j == TJ - 1))
                    nc.vector.tensor_copy(qt[:, qc * 512:(qc + 1) * 512], ps)
            return kt, qt, [lambda c=c: group(c) for c in range(6)]

        def emit_v(kb, vc):
            half, k8 = kb // 8, kb % 8
            ps = ps_s.tile([128, 512], F32, tag="s", name="psv", padded_shape=[128, 1024])
            for j in range(TJ):
                nc.tensor.matmul(
                    ps, tok8_sb[half][j][:, :, k8 * 128:(k8 + 1) * 128],
                    wv8_sb[j][:, :, vc * 512:(vc + 1) * 512],
                    perf_mode=DR, start=(j == 0), stop=(j == TJ - 1))
            psr = ps.rearrange("p (h d) -> p h d", d=DH)
            nc.vector.tensor_copy(va8_sb[kb][:, vc * 8:(vc + 1) * 8, 0:DH], psr)

        def emit_s(p, kb, hi, kt, qt):
            sps = ps_s.tile([128, 1024], F32, tag="s", name="sps")
            hsl = slice(hi * 64, hi * 64 + 64)
            for qc in range(2):
                qsl = slice(qc * 512, (qc + 1) * 512)
                nc.tensor.matmul(sps[:, qsl], kt[hsl, kb * 128:(kb + 1) * 128],
                                 qt[hsl, qsl], start=True, stop=True)
            e = e8_sb[kb][hi]
            nc.scalar.activation(e, sps, mybir.ActivationFunctionType.Exp,
                                 bias=pbias_t, scale=SCALE)
            nc.vector.tensor_mul(e, e, rfT_sb[kb])

        pvst = {}

        def pv_chunk(p, ck):
            # accumulate e8 kb-tiles 2ck, 2ck+1 of pair p into pvs psum
            if ck == 0:
                pvst["pvs"] = [ps_pv.tile([DH + 1, 512], F32, tag="pv",
                                          name="pv") for _ in range(4)]
            pvs = pvst["pvs"]
            for kb in (2 * ck, 2 * ck + 1):
                for hi in range(2):
                    h = 2 * p + hi
                    for qc in range(2):
                        nc.tensor.matmul(
                            pvs[hi * 2 + qc], va8_sb[kb][:, h, :],
                            e8_sb[kb][hi][:, qc * 512:(qc + 1) * 512],
                            start=(kb == 0), stop=(kb == KB - 1))

        def pv_copies():
            aors = []
            for i in range(4):
                a = aorp.tile([DH + 1, 512], F32, tag="aor", name="aor")
                nc.vector.tensor_copy(a, pvst["pvs"][i])
                aors.append(a)
            pvst["aors"] = aors

        def emit_norm_a(aors4):
            srows = srp.tile([4, 512], F32, name="srows")
            for i, a in enumerate(aors4):
                nc.sync.dma_start(out=srows[i:i + 1, :], in_=a[DH:DH + 1, :])
            nc.vector.reciprocal(srows, srows)
            recd = drp.tile([4, 512], F32, tag="recd", name="recd")
            nc.sync.dma_start(out=recd, in_=srows)
            return recd

        def emit_norm_b(p, aors4, recd, which):
            for hi, qc in which:
                i = hi * 2 + qc
                bc = bcp.tile([DH, 512], F32, tag="bc", name="bc")
                nc.gpsimd.dma_start(out=bc,
                                    in_=recd[i].partition_broadcast(DH))
                nc.vector.tensor_mul(
                    aoT8[hi * 64:(hi + 1) * 64, p, qc * 512:(qc + 1) * 512],
                    aors4[i][0:DH, :], bc)

        # ---- schedule ----
        vq = [(kb, vc) for kb in range(KB) for vc in range(2)]
        kt, qt, kq_groups = emit_kq(0)
        for g in (kq_groups[0], kq_groups[4], kq_groups[5]):
            g()   # K chunk 0 + both Q chunks: enough for S(0, kb 0-3)
        kq0_rest = {2: kq_groups[1], 5: kq_groups[2], 7: kq_groups[3]}
        wp8_sb = []
        prev = {}
        for p in range(NPAIR):
            nxt = None
            last = p == NPAIR - 1
            for kb in range(KB):
                if p >= 1 and kb < 8:
                    pv_chunk(p - 1, kb)
                elif last and 9 <= kb < 15:
                    pv_chunk(p, kb - 9)
                emit_s(p, kb, 0, kt, qt)
                emit_s(p, kb, 1, kt, qt)
                if p == 0:
                    if kb in kq0_rest:
                        kq0_rest[kb]()
                    for _ in range(2):  # V filler
                        if vq:
                            emit_v(*vq.pop(0))
                if p >= 1:
                    if kb == 8:
                        pv_copies()
                    elif kb == 9:
                        prev["recd"] = emit_norm_a(pvst["aors"])
                    elif kb == 10:
                        emit_norm_b(p - 1, pvst["aors"], prev["recd"],
                                    [(0, 0), (0, 1)])
                    elif kb == 11:
                        emit_norm_b(p - 1, pvst.pop("aors"), prev.pop("recd"),
                                    [(1, 0), (1, 1)])
                if kb >= 10 and p < NPAIR - 1:
                    if nxt is None:
                        nxt = emit_kq(p + 1)
                    nxt[2][kb - 10]()
                if p == 1 and kb < 4:  # wp8 into freed wv slots after V phase
                    t = p_wv.tile([128, 2, 1024], F8, tag="wv8", name="wpt")
                    nc.sync.dma_start(
                        out=t, in_=wp8[kb].rearrange("p (i n) -> p i n", i=2))
                    wp8_sb.append(t)
            if nxt is not None:
                kt, qt = nxt[0], nxt[1]
        pv_chunk(NPAIR - 1, 6)
        pv_chunk(NPAIR - 1, 7)
        pv_copies()
        recd = emit_norm_a(pvst["aors"])
        emit_norm_b(NPAIR - 1, pvst["aors"], recd, [(0, 0), (0, 1)])
        emit_norm_b(NPAIR - 1, pvst.pop("aors"), recd, [(1, 0), (1, 1)])

        # ---- projection + residual + layernorm ----
        resid_sb = {}

        def load_resid(rb):
            t = rsp.tile([128, 1024], BF16, tag="resid", name="rt")
            nc.sync.dma_start(out=t, in_=resid[rb * 128:(rb + 1) * 128, :])
            resid_sb[rb] = t

        load_resid(0)
        load_resid(1)
        load_resid(2)
        for rb in range(8):
            rsl = slice(rb * 128, (rb + 1) * 128)
            x = xp.tile([128, 1024], F32, tag="x")
            for nch in range(2):
                nsl = slice(nch * 512, (nch + 1) * 512)
                psy = ps_s.tile([128, 512], F32, tag="s", name="psy", padded_shape=[128, 1024])
                for jq in range(4):
                    nc.tensor.matmul(psy, aoT8[:, 2 * jq:2 * jq + 2, rsl],
                                     wp8_sb[jq][:, :, nsl],
                                     perf_mode=DR, start=(jq == 0), stop=(jq == 3))
                nc.vector.scalar_tensor_tensor(
                    x[:, nsl], psy, XSCL, resid_sb[rb][:, nsl],
                    op0=mybir.AluOpType.mult, op1=mybir.AluOpType.add)
            st = smp.tile([128, 2, 6], F32, tag="st")
            for s2 in range(2):
                nc.vector.bn_stats(st[:, s2, :], x[:, s2 * 512:(s2 + 1) * 512])
            mv = smp.tile([128, 2], F32, tag="mv")
            nc.vector.bn_aggr(mv, st)
            rstd = smp.tile([128, 1], F32, tag="small")
            nc.scalar.activation(rstd, mv[:, 1:2],
                                 mybir.ActivationFunctionType.Sqrt, bias=eps_t)
            nc.vector.reciprocal(rstd, rstd)
            nmr = smp.tile([128, 1], F32, tag="small")
            nc.vector.tensor_mul(nmr, mv[:, 0:1], rstd)
            nc.vector.tensor_scalar_mul(nmr, nmr, -1.0)
            nc.scalar.activation(x, x, mybir.ActivationFunctionType.Identity,
                                 bias=nmr, scale=rstd)
            if not skip_affine:
                nc.vector.tensor_mul(x, x, gamma_b)
                nc.vector.tensor_add(x, x, lnb_b)
            nc.sync.dma_start(out=out[rsl, :], in_=x)
            if rb + 3 < 8:
                load_resid(rb + 3)


_CACHE = {}


def _build(skip_affine):
    key = bool(skip_affine)
    if key in _CACHE:
        return _CACHE[key]
    nc = bacc.Bacc("TRN2", target_bir_lowering=False, debug=False,
                   num_devices=NCORE)
    aps = (
        nc.dram_tensor("tok8", [TJ, 128, 2 * L], F8, kind="ExternalInput").ap(),
        nc.dram_tensor("wq8", [128, 8192], F8, kind="ExternalInput").ap(),
        nc.dram_tensor("wk8", [128, 8192], F8, kind="ExternalInput").ap(),
        nc.dram_tensor("wv8", [TJ, 128, 2 * D], F8, kind="ExternalInput").ap(),
        nc.dram_tensor("wp8", [TJ, 128, 2 * D], F8, kind="ExternalInput").ap(),
        nc.dram_tensor("rfT", [L, LQ], BF16, kind="ExternalInput").ap(),
        nc.dram_tensor("resid", [LQ, D], BF16, kind="ExternalInput").ap(),
        nc.dram_tensor("gamma", [D], BF16, kind="ExternalInput").ap(),
        nc.dram_tensor("lnb", [D], BF16, kind="ExternalInput").ap(),
        nc.dram_tensor("out", [LQ, D], F32, kind="ExternalOutput").ap(),
    )
    with tile.TileContext(nc) as tc:
        _emit(tc, aps, skip_affine)
    nc.compile()
    _CACHE[key] = nc
    return nc


def _dlayout(M):
    """[D, N] -> [TJ, 128, 2, N] fp8: D-dim index = j*256 + i*128 + p."""
    N = M.shape[1]
    return np.ascontiguousarray(
        M.reshape(TJ, 2, 128, N).transpose(0, 2, 1, 3).astype(NPF8))


def kernel(tokens, R, Wq, Wk, Wv, Wp, beta, gamma, bias, _spmd_kwargs=None):
    tokens = np.asarray(tokens, dtype=np.float32)
    R = np.asarray(R, dtype=np.float32)
    skip_affine = bool(np.all(gamma == 1.0) and np.all(bias == 0.0))
    nc = _build(skip_affine)

    rfac = np.maximum(R, EPS_R).astype(np.float64) ** float(beta[0])
    rfacT = rfac.T.astype(NPBF16)  # [keys, queries]
    wq8 = np.ascontiguousarray(
        Wq.T.reshape(TJ, 2, 128, NPAIR, 128).transpose(2, 3, 0, 1, 4)
        .astype(NPF8)).reshape(128, 8192)
    wk8 = np.ascontiguousarray(
        Wk.T.reshape(TJ, 2, 128, NPAIR, 128).transpose(2, 3, 0, 1, 4)
        .astype(NPF8)).reshape(128, 8192)
    wv8 = _dlayout(np.ascontiguousarray(Wv.T)).reshape(TJ, 128, 2 * D)
    # wp8 contraction index jq*256 + i*128 + p -> head 2*(2jq+i) + p//64,
    # dh = p%64, i.e. row (head*64 + dh) of Wp.T; stored x16.
    wp8 = np.ascontiguousarray(
        (Wp.T * WPSCL).reshape(TJ, 2, 2, DH, D)   # [jq, i, hpar, dh, n]
        .transpose(0, 2, 3, 1, 4)                 # [jq, hpar, dh, i, n]
        .astype(NPF8)).reshape(TJ, 128, 2 * D)
    gamma_b = gamma.astype(NPBF16)
    lnb_b = bias.astype(NPBF16)

    in_maps = []
    for c in range(NCORE):
        b, q0 = c // 2, (c % 2) * LQ
        tokT = tokens[b].T  # [D, L]
        rot = np.concatenate(
            [tokT[:, q0:q0 + LQ], tokT[:, LQ - q0:2 * LQ - q0]], axis=1)
        in_maps.append({
            "tok8": _dlayout(np.ascontiguousarray(rot)).reshape(TJ, 128, 2 * L),
            "wq8": wq8, "wk8": wk8, "wv8": wv8, "wp8": wp8,
            "rfT": np.ascontiguousarray(np.concatenate(
                [rfacT[q0:q0 + LQ, q0:q0 + LQ],
                 rfacT[LQ - q0:2 * LQ - q0, q0:q0 + LQ]], axis=0)),
            "resid": np.ascontiguousarray(
                tokens[b, q0:q0 + LQ, :].astype(NPBF16)),
            "gamma": gamma_b, "lnb": lnb_b,
        })

    res = run_bass_kernel_spmd(nc, in_maps, core_ids=list(range(NCORE)),
                               **(_spmd_kwargs or {}))
    out = np.empty((B, L, D), dtype=np.float32)
    for c in range(NCORE):
        b, q0 = c // 2, (c % 2) * LQ
        out[b, q0:q0 + LQ, :] = res.results[c]["out"]
    if _spmd_kwargs:
        kernel._last_result = res
    return out
